# revision 11
# baseline (speedup 1.0000x reference)
"""Trainium2 Bass kernel for nn_AbstractLinear (linear forward + interval
bound propagation).

kernel(x, low, high, W, b) -> (y, low_out, high_out)
  y        = x @ W.T + b                       [4096, 16384] fp32
  low_out  = W @ c + b - |W| @ r               [16384] fp32
  high_out = W @ c + b + |W| @ r               [16384] fp32
  with c = (low+high)/2, r = (high-low)/2.

Sharding: column-parallel over the 16384 output features across 8 cores
(2048 per core); x/low/high replicated; no collectives; outputs
concatenated on host.

Per-core device kernel:
  - x and the W shard are converted to fp16 on the host and pre-tiled so
    every DMA is a contiguous block. fp16 keeps an 11-bit mantissa — the
    same precision class as the PE's tf32/f32r mode (which rounds fp32
    operands to 11-bit mantissa anyway) — while halving DMA bytes, which
    is what actually limits this kernel on hardware. All accumulation is
    fp32 (PSUM / DVE).
  - The whole W shard (16MB fp16) stays SBUF-resident; x streams through
    once as [128, XCH, 128] stationary tiles; each k-tile issues 4
    N=512 matmuls accumulating a [128, 2048] fp32 PSUM block per batch
    tile (PE roofline for this shape: ~873us/core).
  - IBP is fused into the W load: ScalarE computes |r_k*W| (fp16->fp32),
    VectorE accumulates rad and (fused multiply-add) mid per partition;
    an all-ones 128x128 fp32 matmul reduces across partitions at the end
    (M=1 matmuls fail NEFF load, hence the full-width ones reduce).
"""

import numpy as np

BATCH = 4096
IN = 4096
OUT = 16384
N_CORES = 8
O_SHARD = OUT // N_CORES  # 2048
N_K = IN // 128           # 32 contraction tiles
N_M = BATCH // 128        # 32 batch tiles
NCHUNK = 512              # matmul moving free dim / PSUM bank width

_CACHE = {}


def _build_nc(reps=1):
    import concourse.bacc as bacc
    import concourse.mybir as mybir
    import concourse.tile as tile

    f32 = mybir.dt.float32
    f16 = mybir.dt.float16
    Alu = mybir.AluOpType
    Act = mybir.ActivationFunctionType

    nc = bacc.Bacc(
        "TRN2",
        target_bir_lowering=False,
        debug=False,
        enable_asserts=False,
    )

    # host-tiled fp16 layouts; every DMA is a contiguous block:
    #   xt [k, m, p, b] = x.T.reshape(N_K,128,N_M,128).transpose(0,2,1,3)
    #   wt [k, p, o]    = W_shard.T.reshape(N_K,128,O_SHARD)
    xt_d = nc.dram_tensor("xt", [N_K, N_M, 128, 128], f16, kind="ExternalInput")
    wt_d = nc.dram_tensor("wt", [N_K, 128, O_SHARD], f16, kind="ExternalInput")
    b_d = nc.dram_tensor("b", [O_SHARD], f32, kind="ExternalInput")
    c_d = nc.dram_tensor("c", [IN], f32, kind="ExternalInput")
    r_d = nc.dram_tensor("r", [IN], f32, kind="ExternalInput")
    y_d = nc.dram_tensor("y", [BATCH, O_SHARD], f32, kind="ExternalOutput")
    lo_d = nc.dram_tensor("lo", [O_SHARD], f32, kind="ExternalOutput")
    hi_d = nc.dram_tensor("hi", [O_SHARD], f32, kind="ExternalOutput")

    with tile.TileContext(nc) as tc:
        with (
            tc.tile_pool(name="wt", bufs=1) as wt_pool,
            tc.tile_pool(name="xt", bufs=2) as xt_pool,
            tc.tile_pool(name="ysb", bufs=2) as y_pool,
            tc.tile_pool(name="misc", bufs=1) as misc_pool,
            tc.tile_pool(name="ibp", bufs=1) as ibp_pool,
            tc.tile_pool(name="psum", bufs=2, space="PSUM") as psum_pool,
        ):
            # center/radius laid out [p, k] with i = k*128 + p
            c_sb = misc_pool.tile([128, N_K], f32, tag="c")
            nc.sync.dma_start(out=c_sb, in_=c_d.ap().rearrange("(k p) -> p k", p=128))
            r_sb = misc_pool.tile([128, N_K], f32, tag="r")
            nc.sync.dma_start(out=r_sb, in_=r_d.ap().rearrange("(k p) -> p k", p=128))
            # all-ones stationary for partition-sum matmuls (M=1 fails NEFF load)
            ones_sb = misc_pool.tile([128, 128], f32, tag="ones")
            nc.vector.memset(ones_sb, 1.0)

            for rep in range(reps):
                b_bc = ibp_pool.tile([128, O_SHARD], f32, tag="bbc")
                nc.gpsimd.dma_start(
                    out=b_bc,
                    in_=b_d.ap().unsqueeze(0).to_broadcast([128, O_SHARD]),
                )

                acc_mid = ibp_pool.tile([128, O_SHARD], f32, tag="amid")
                nc.vector.memset(acc_mid, 0.0)
                acc_rad = ibp_pool.tile([128, O_SHARD], f32, tag="arad")
                nc.vector.memset(acc_rad, 0.0)

                wts = []
                for k in range(N_K):
                    t = wt_pool.tile([128, O_SHARD], f16, tag=f"wt{k}")
                    nc.sync.dma_start(out=t, in_=wt_d.ap()[k])
                    wts.append(t)

                XCH = 4  # k-tiles per xt chunk
                NH = O_SHARD // NCHUNK
                for m in range(N_M):
                    msl = slice(m * 128, (m + 1) * 128)
                    pss = [
                        psum_pool.tile([128, NCHUNK], f32, tag=f"ps{h}", name=f"ps{h}_{m}")
                        for h in range(NH)
                    ]
                    for kc in range(0, N_K, XCH):
                        xt_t = xt_pool.tile([128, XCH, 128], f16, tag="xt", bufs=10)
                        nc.sync.dma_start(
                            out=xt_t,
                            in_=xt_d.ap()[kc : kc + XCH, m].rearrange("k p b -> p k b"),
                        )
                        for k in range(kc, kc + XCH):
                            st, sp = (k == 0), (k == N_K - 1)
                            lhs = xt_t[:, k - kc, :]
                            for h in range(NH):
                                nc.tensor.matmul(
                                    pss[h],
                                    lhs,
                                    wts[k][:, h * NCHUNK : (h + 1) * NCHUNK],
                                    start=st,
                                    stop=sp,
                                )
                    for h in range(NH):
                        hsl = slice(h * NCHUNK, (h + 1) * NCHUNK)
                        y_sb = y_pool.tile([128, NCHUNK], f32, tag="ysb", bufs=4)
                        nc.vector.tensor_tensor(y_sb, pss[h], b_bc[:, hsl], Alu.add)
                        nc.sync.dma_start(out=y_d.ap()[msl, hsl], in_=y_sb)
                    # IBP for k-tile m, spread across the batch loop
                    # (acc_mid += c_k * W, acc_rad += |r_k * W|; fp16 -> fp32)
                    k = m
                    t = wts[k]
                    for u in range(2):
                        usl = slice(u * (O_SHARD // 2), (u + 1) * (O_SHARD // 2))
                        tmpa = ibp_pool.tile([128, O_SHARD // 2], f32, tag="tmpa")
                        nc.scalar.activation(
                            tmpa, t[:, usl], Act.Abs, scale=r_sb[:, k : k + 1]
                        )
                        nc.vector.tensor_tensor(
                            acc_rad[:, usl], acc_rad[:, usl], tmpa, Alu.add
                        )
                        nc.vector.scalar_tensor_tensor(
                            acc_mid[:, usl],
                            t[:, usl],
                            c_sb[:, k : k + 1],
                            acc_mid[:, usl],
                            Alu.mult,
                            Alu.add,
                        )

                # partition-reduce IBP accumulators, finish lo/hi
                red_m = [
                    psum_pool.tile([128, NCHUNK], f32, tag=f"ps{h}", name=f"redm{h}") for h in range(NH)
                ]
                red_r = [
                    psum_pool.tile([128, NCHUNK], f32, tag=f"ps{h}", name=f"redr{h}") for h in range(NH)
                ]
                for h in range(NH):
                    hsl = slice(h * NCHUNK, (h + 1) * NCHUNK)
                    nc.tensor.matmul(red_m[h], ones_sb, acc_mid[:, hsl])
                    nc.tensor.matmul(red_r[h], ones_sb, acc_rad[:, hsl])
                for h in range(NH):
                    hsl = slice(h * NCHUNK, (h + 1) * NCHUNK)
                    mid_sb = misc_pool.tile([1, NCHUNK], f32, tag="msb")
                    nc.vector.tensor_tensor(
                        mid_sb, red_m[h][0:1, :], b_bc[0:1, hsl], Alu.add
                    )
                    lo_sb = misc_pool.tile([1, NCHUNK], f32, tag="losb")
                    nc.vector.tensor_tensor(lo_sb, mid_sb, red_r[h][0:1, :], Alu.subtract)
                    hi_sb = misc_pool.tile([1, NCHUNK], f32, tag="hisb")
                    nc.vector.tensor_tensor(hi_sb, mid_sb, red_r[h][0:1, :], Alu.add)
                    # 1-D DMA APs fail NEFF load; keep both sides 2-D [1, N]
                    nc.sync.dma_start(out=lo_d.ap()[hsl].unsqueeze(0), in_=lo_sb)
                    nc.sync.dma_start(out=hi_d.ap()[hsl].unsqueeze(0), in_=hi_sb)

    nc.compile()
    return nc


def kernel(x, low, high, W, b, _trace=False):
    from concourse.bass_utils import run_bass_kernel_spmd

    if "nc" not in _CACHE:
        _CACHE["nc"] = _build_nc()
    nc = _CACHE["nc"]

    x = np.asarray(x, dtype=np.float32)
    low = np.asarray(low, dtype=np.float32)
    high = np.asarray(high, dtype=np.float32)
    W = np.asarray(W, dtype=np.float32)
    b = np.asarray(b, dtype=np.float32)

    xt = np.ascontiguousarray(
        x.astype(np.float16).T.reshape(N_K, 128, N_M, 128).transpose(0, 2, 1, 3)
    )
    c = (low + high) * np.float32(0.5)
    r = (high - low) * np.float32(0.5)

    in_maps = []
    for core in range(N_CORES):
        s = slice(core * O_SHARD, (core + 1) * O_SHARD)
        wtt = np.ascontiguousarray(W[s].astype(np.float16).T).reshape(
            N_K, 128, O_SHARD
        )
        in_maps.append(
            {
                "xt": xt,
                "wt": wtt,
                "b": np.ascontiguousarray(b[s]),
                "c": c,
                "r": r,
            }
        )

    bres = run_bass_kernel_spmd(nc, in_maps, list(range(N_CORES)), trace=_trace)
    _CACHE["last"] = bres
    res = bres.results
    y = np.concatenate([res[i]["y"] for i in range(N_CORES)], axis=1)
    lo = np.concatenate([res[i]["lo"] for i in range(N_CORES)])
    hi = np.concatenate([res[i]["hi"] for i in range(N_CORES)])
    return (y, lo, hi)
